# revision 20
# baseline (speedup 1.0000x reference)
"""Trainium2 Bass kernel for a fused LSTM cell.

Reference math (B=8192, D=U=1024, all fp32):
    z = x @ Wx + h_tm1 @ Uh + b          # Wx=[W_i|W_f|W_c|W_o], Uh likewise
    i, f = sigmoid(z_i), sigmoid(z_f)
    c = f * c_tm1 + i * tanh(z_c)
    h = sigmoid(z_o) * tanh(c)
    returns (h, c)

Strategy:
  - Data-parallel over 8 NeuronCores: batch 8192 -> 1024 rows/core,
    weights replicated. No collectives.
  - Per core the GEMM is computed transposed: z^T [4096 units, 1024 batch].
    lhsT (stationary) = weight tiles [128k, 128n]; rhs (moving) =
    host-pretransposed [x|h]^T tiles [128k, 512 batch]. Units on PSUM
    partitions so the per-unit bias folds into the ScalarE activation.
  - GEMM operands in fp16 (PSUM accumulation stays fp32): same 1 col/cycle
    PE rate as fp32r but half the HBM traffic, which makes the j=0 ramp
    PE-bound instead of DMA-bound, and 2-byte weights get the fast
    weight-load path. Quantization error ~2.3e-3 rel (vs 2e-2 budget).
  - j=0 runs ko-major over 6 PSUM groups so the PE chases the arriving
    xh/w stream; its last 2 groups (the c~ gate) run as a second wave so
    j=1 never waits for all 8 PSUM banks at once.
  - Gate order f,i,o,c~ in the steady-state blocks; the very last group
    runs c~,f,i,o so everything except act(o) -> h mul -> DMA overlaps
    the o-gate matmuls. Outputs ship as fp16 (written directly by the
    producing vector ops; host upcasts) to halve output HBM traffic.
"""

from contextlib import ExitStack

import numpy as np

import concourse.tile as tile
from concourse import bacc, mybir
from concourse.bass_utils import run_bass_kernel_spmd

B, D, U = 8192, 1024, 1024
NCORES = 8
BS = B // NCORES  # per-core batch rows
F = 512           # moving-operand cols per matmul (one PSUM bank of fp32)


def build_nc(bs=BS, d=D, u=U):
    """Build the per-core SPMD Bass program.

    DRAM parameter layouts (host prepares these):
      xh   [KO, 128, bs] fp16   : [x|h]^T, contraction on (KO, partition)
      w    [JB, KO2, 128, 2, 4, 128] fp16 :
           w[j,ko2,p,e,g,n] = W_all[(ko2*2+e)*128+p, (g*JB+j)*128+n]
      bias [128, NT] fp32       : bias[p, t] = b_all[t*128+p]
      ct   [JB, 128, bs] fp32   : c_tm1^T unit-blocks
      h_out/c_out [JB, 128, bs] fp16 : h^T / c^T unit-blocks
    """
    kdim = d + u
    KO = kdim // 128    # contraction 128-blocks
    KO2 = KO // 2       # two contraction blocks per weight tile (2KB lines)
    JB = u // 128       # unit blocks per gate
    NT = 4 * u // 128   # total n-tiles (4 gates)
    BH = bs // F        # moving chunks per xh tile

    f32 = mybir.dt.float32
    f16 = mybir.dt.float16
    SIG = mybir.ActivationFunctionType.Sigmoid
    TANH = mybir.ActivationFunctionType.Tanh

    nc = bacc.Bacc("TRN2", target_bir_lowering=False, debug=False)

    xh = nc.dram_tensor("xh", [KO, 128, bs], f16, kind="ExternalInput").ap()
    w = nc.dram_tensor("w", [JB, KO2, 128, 2, 4, 128], f16, kind="ExternalInput").ap()
    bia = nc.dram_tensor("bias", [128, NT], f32, kind="ExternalInput").ap()
    ct = nc.dram_tensor("ct", [JB, 128, bs], f32, kind="ExternalInput").ap()
    ho = nc.dram_tensor("h_out", [JB, 128, bs], f16, kind="ExternalOutput").ap()
    co = nc.dram_tensor("c_out", [JB, 128, bs], f16, kind="ExternalOutput").ap()

    # f gate first so t1 = f*ct can issue early (the very last group instead
    # uses c~,f,i,o so only act(o) -> h mul -> DMA trails the final matmul)
    G_ORDER = (1, 0, 3, 2)

    with tile.TileContext(nc) as tc, ExitStack() as ctx:
        xh_pool = ctx.enter_context(tc.tile_pool(name="xh", bufs=1))
        w_pool = ctx.enter_context(tc.tile_pool(name="w", bufs=2 * KO2))
        bias_pool = ctx.enter_context(tc.tile_pool(name="bias", bufs=1))
        ct_pool = ctx.enter_context(tc.tile_pool(name="ct", bufs=2))
        gate_pool = ctx.enter_context(tc.tile_pool(name="gates", bufs=2))
        out_pool = ctx.enter_context(tc.tile_pool(name="outs", bufs=2))
        psum_pool = ctx.enter_context(tc.tile_pool(name="psum", bufs=8, space="PSUM"))

        bias_sb = bias_pool.tile([128, NT], f32, tag="bias")
        nc.sync.dma_start(bias_sb[:], bia[:])

        # HAM warm-up: ~4us of dependency-free matmuls on garbage SBUF while
        # the first real tiles are still in flight, so the PE clock gate is
        # already at 8/8 when the real stream starts.
        warm_sb = gate_pool.tile([128, 128], f16, tag="warm")
        nc.vector.memset(warm_sb[:], 0.0)
        warm_ps = psum_pool.tile([128, F], f32, tag="ps", name="warm_ps")
        for _ in range(85):
            nc.tensor.matmul(
                warm_ps[:, :64], lhsT=warm_sb[:], rhs=warm_sb[:, :64],
                start=True, stop=True,
            )

        def load_ct(j):
            t = ct_pool.tile([128, bs], f32, tag="ct")
            nc.sync.dma_start(t[:], ct[j])
            return t

        def load_wk(j, ko2):
            t = w_pool.tile([128, 2, 4, 128], f16, tag="wk", name=f"wk_{j}_{ko2}")
            nc.sync.dma_start(t[:], w[j, ko2])
            return t

        # Startup: interleave j=0 weights with the xh stream so the PE can
        # start after the first ~0.5MB instead of the full prefix.
        xh_sb = []
        wk_by_j = {0: []}
        ct_by_j = {}
        for ko2 in range(KO2):
            wk_by_j[0].append(load_wk(0, ko2))
            for e in range(2):
                t = xh_pool.tile([128, bs], f16, tag=f"xh{2 * ko2 + e}")
                nc.sync.dma_start(t[:], xh[2 * ko2 + e])
                xh_sb.append(t)
            if ko2 == 2:
                # after the 3rd weight/xh pair: early enough to beat j=0's
                # epilogue, late enough not to stall the PE's ko chase
                ct_by_j[0] = load_ct(0)

        def act_gate(j, g, ps, gtile=None, gsl=slice(0, F)):
            if gtile is None:
                gtile = gate_pool.tile([128, F], f32, tag=f"g{g}")
            idx = g * JB + j
            func = TANH if g == 2 else SIG
            nc.scalar.activation(
                gtile[:, gsl], ps[:, gsl], func, bias=bias_sb[:, idx : idx + 1]
            )
            return gtile

        def epilogue(j, bh, gt, ct_sb, h_out, c_out, do_dma=False):
            gsl = slice(0, F)
            bsl = slice(bh * F, (bh + 1) * F)
            t1 = gate_pool.tile([128, F], f32, tag="t1")
            nc.vector.tensor_mul(t1[:], gt[1][:, gsl], ct_sb[:, bsl])
            t2 = gate_pool.tile([128, F], f32, tag="t2")
            nc.vector.tensor_mul(t2[:], gt[0][:, gsl], gt[2][:, gsl])
            nc.vector.tensor_add(c_out[:, bsl], t1[:], t2[:])
            tct = gate_pool.tile([128, F], f32, tag="tct")
            nc.scalar.activation(tct[:], c_out[:, bsl], TANH)
            nc.vector.tensor_mul(h_out[:, bsl], gt[3][:, gsl], tct[:])
            if do_dma:
                nc.sync.dma_start(ho[j][:, bsl], h_out[:, bsl])
                nc.scalar.dma_start(co[j][:, bsl], c_out[:, bsl])

        for j in range(JB):
            if j + 1 < JB:
                wk_by_j[j + 1] = [load_wk(j + 1, ko2) for ko2 in range(KO2)]
                ct_by_j[j + 1] = load_ct(j + 1)
            wk = wk_by_j.pop(j)
            ct_sb = ct_by_j.pop(j)
            h_out = out_pool.tile([128, bs], f16, tag="h")
            c_out = out_pool.tile([128, bs], f16, tag="c")
            gt = {}
            if j == 0:
                # ko-major chase in two waves: 7 groups while xh streams in
                # (max cushion against early-DMA jitter), then the last c~
                # group, so j=1's first group only waits on the first
                # wave-1 activation.
                wave1 = [(g, bh) for g in G_ORDER[:3] for bh in range(BH)]
                wave1.append((2, 0))
                ps = {
                    g: [
                        psum_pool.tile([128, F], f32, tag="ps", name=f"ps0_{g}_{bh}")
                        for bh in range(BH)
                    ]
                    for g in G_ORDER
                }
                for ko in range(KO):
                    for g, bh in wave1:
                        nc.tensor.matmul(
                            ps[g][bh][:],
                            lhsT=wk[ko // 2][:, ko % 2, g, :],
                            rhs=xh_sb[ko][:, bh * F : (bh + 1) * F],
                            start=(ko == 0),
                            stop=(ko == KO - 1),
                        )
                for ko in range(KO):
                    nc.tensor.matmul(
                        ps[2][1][:],
                        lhsT=wk[ko // 2][:, ko % 2, 2, :],
                        rhs=xh_sb[ko][:, F : 2 * F],
                        start=(ko == 0),
                        stop=(ko == KO - 1),
                    )
                for g in G_ORDER:
                    gb = []
                    for bh in range(BH):
                        gtile = act_gate(j, g, ps[g][bh])
                        gb.append(gtile)
                    gt[g] = gb
                for bh in range(BH):
                    epilogue(j, bh, [gt[g][bh] for g in range(4)], ct_sb, h_out, c_out)
                nc.sync.dma_start(ho[j][:], h_out[:])
                nc.scalar.dma_start(co[j][:], c_out[:])
            else:
                # bh-major: bh=0's gates+epilogue fully overlap bh=1's
                # matmuls, so only bh=1's final chain trails the last MM.
                last = j == JB - 1
                for bh in range(BH):
                    # Last group of the whole kernel: gate order c~,f,i,o so
                    # c (and its DMA) completes during the o-gate matmuls and
                    # the post-last-MM chain is just act(o) -> h mul -> DMA.
                    gorder = (2, 1, 0, 3) if (last and bh == BH - 1) else G_ORDER
                    gtb = {}
                    for g in gorder:
                        psb = psum_pool.tile(
                            [128, F], f32, tag="ps", name=f"ps_{g}_{bh}"
                        )
                        for ko in range(KO):
                            nc.tensor.matmul(
                                psb[:],
                                lhsT=wk[ko // 2][:, ko % 2, g, :],
                                rhs=xh_sb[ko][:, bh * F : (bh + 1) * F],
                                start=(ko == 0),
                                stop=(ko == KO - 1),
                            )
                        if last and bh == BH - 1 and g == 3:
                            gtb[g] = psb  # act chunked below, after the chain
                        else:
                            gtb[g] = act_gate(j, g, psb)
                        if last and bh == BH - 1 and g == 0:
                            # c-chain runs here, overlapped by the o-gate MMs
                            bsl = slice(bh * F, (bh + 1) * F)
                            t1 = gate_pool.tile([128, F], f32, tag="t1")
                            nc.vector.tensor_mul(t1[:], gtb[1][:], ct_sb[:, bsl])
                            t2 = gate_pool.tile([128, F], f32, tag="t2")
                            nc.vector.tensor_mul(t2[:], gtb[0][:], gtb[2][:])
                            nc.vector.tensor_add(c_out[:, bsl], t1[:], t2[:])
                            nc.scalar.dma_start(co[j][:, bsl], c_out[:, bsl])
                            tct = gate_pool.tile([128, F], f32, tag="tct")
                            nc.scalar.activation(tct[:], c_out[:, bsl], TANH)
                    if not (last and bh == BH - 1):
                        epilogue(
                            j, bh, [gtb[g] for g in range(4)], ct_sb, h_out, c_out,
                            do_dma=(last and bh == 0),
                        )
                        if not last and bh == BH - 1:
                            nc.sync.dma_start(ho[j][:], h_out[:])
                            nc.scalar.dma_start(co[j][:], c_out[:])
                    else:
                        # post-last-MM: act(o) + h mul + DMA in 256-col chunks
                        oo = gate_pool.tile([128, F], f32, tag="g3b")
                        for ci in range(2):
                            wd = F // 2
                            gsl = slice(ci * wd, (ci + 1) * wd)
                            act_gate(j, 3, gtb[3], gtile=oo, gsl=gsl)
                            bsl = slice(bh * F + ci * wd, bh * F + (ci + 1) * wd)
                            nc.vector.tensor_mul(
                                h_out[:, bsl], oo[:, gsl], tct[:, gsl]
                            )
                            nc.sync.dma_start(ho[j][:, bsl], h_out[:, bsl])

    nc.compile()
    return nc


def pack_shared(inputs):
    """Weight + bias device arrays (replicated on every core)."""
    d, u = inputs["W_i"].shape[0], inputs["W_i"].shape[1]
    kdim = d + u
    KO = kdim // 128
    KO2 = KO // 2
    NT = 4 * u // 128
    JB = u // 128
    Wx = np.concatenate(
        [inputs["W_i"], inputs["W_f"], inputs["W_c"], inputs["W_o"]], axis=1
    )
    Uh = np.concatenate(
        [inputs["U_i"], inputs["U_f"], inputs["U_c"], inputs["U_o"]], axis=1
    )
    W_all = np.concatenate([Wx, Uh], axis=0)  # [kdim, 4u]
    # w_dev[j, ko2, p, e, g, n] = W_all[(ko2*2+e)*128+p, (g*JB+j)*128+n]
    w_dev = np.ascontiguousarray(
        W_all.reshape(KO2, 2, 128, 4, JB, 128).transpose(4, 0, 2, 1, 3, 5)
    ).astype(np.float16)
    b_all = np.concatenate(
        [inputs["b_i"], inputs["b_f"], inputs["b_c"], inputs["b_o"]]
    )  # [4u]
    b_dev = np.ascontiguousarray(b_all.reshape(NT, 128).T).astype(np.float32)
    return w_dev, b_dev


def pack_core(x_i, h_i, c_i):
    """Per-core shard arrays."""
    bs = x_i.shape[0]
    d, u = x_i.shape[1], h_i.shape[1]
    KO = (d + u) // 128
    JB = u // 128
    xh_t = np.concatenate([x_i, h_i], axis=1).T  # [kdim, bs]
    xh_dev = np.ascontiguousarray(xh_t.reshape(KO, 128, bs)).astype(np.float16)
    ct_dev = np.ascontiguousarray(c_i.T.reshape(JB, 128, bs)).astype(np.float32)
    return xh_dev, ct_dev


_NC_CACHE = {}


def _get_nc():
    key = (BS, D, U)
    if key not in _NC_CACHE:
        _NC_CACHE[key] = build_nc()
    return _NC_CACHE[key]


def _run(inputs, trace=False):
    x = np.asarray(inputs["inputs"], np.float32)
    h = np.asarray(inputs["h_tm1"], np.float32)
    c = np.asarray(inputs["c_tm1"], np.float32)
    w_dev, b_dev = pack_shared(inputs)
    in_maps = []
    for i in range(NCORES):
        sl = slice(i * BS, (i + 1) * BS)
        xh_dev, ct_dev = pack_core(x[sl], h[sl], c[sl])
        in_maps.append({"xh": xh_dev, "w": w_dev, "bias": b_dev, "ct": ct_dev})
    nc = _get_nc()
    res = run_bass_kernel_spmd(nc, in_maps, list(range(NCORES)), trace=trace)
    u = U
    h_full = np.empty((B, u), np.float32)
    c_full = np.empty((B, u), np.float32)
    for i in range(NCORES):
        sl = slice(i * BS, (i + 1) * BS)
        h_full[sl] = res.results[i]["h_out"].astype(np.float32).reshape(u, BS).T
        c_full[sl] = res.results[i]["c_out"].astype(np.float32).reshape(u, BS).T
    return (h_full, c_full), res


def kernel(**inputs):
    out, _ = _run(inputs, trace=False)
    return out


# revision 21
# speedup vs baseline: 1.0025x; 1.0025x over previous
"""Trainium2 Bass kernel for a fused LSTM cell.

Reference math (B=8192, D=U=1024, all fp32):
    z = x @ Wx + h_tm1 @ Uh + b          # Wx=[W_i|W_f|W_c|W_o], Uh likewise
    i, f = sigmoid(z_i), sigmoid(z_f)
    c = f * c_tm1 + i * tanh(z_c)
    h = sigmoid(z_o) * tanh(c)
    returns (h, c)

Strategy:
  - Data-parallel over 8 NeuronCores: batch 8192 -> 1024 rows/core,
    weights replicated. No collectives.
  - Per core the GEMM is computed transposed: z^T [4096 units, 1024 batch].
    lhsT (stationary) = weight tiles [128k, 128n]; rhs (moving) =
    host-pretransposed [x|h]^T tiles [128k, 512 batch]. Units on PSUM
    partitions so the per-unit bias folds into the ScalarE activation.
  - GEMM operands in fp16 (PSUM accumulation stays fp32): same 1 col/cycle
    PE rate as fp32r but half the HBM traffic, which makes the j=0 ramp
    PE-bound instead of DMA-bound, and 2-byte weights get the fast
    weight-load path. Quantization error ~2.3e-3 rel (vs 2e-2 budget).
  - j=0 runs ko-major over 6 PSUM groups so the PE chases the arriving
    xh/w stream; its last 2 groups (the c~ gate) run as a second wave so
    j=1 never waits for all 8 PSUM banks at once.
  - Gate order f,i,o,c~ in the steady-state blocks; the very last group
    runs c~,f,i,o so everything except act(o) -> h mul -> DMA overlaps
    the o-gate matmuls. Outputs ship as fp16 (written directly by the
    producing vector ops; host upcasts) to halve output HBM traffic.
"""

from contextlib import ExitStack

import numpy as np

import concourse.tile as tile
from concourse import bacc, mybir
from concourse.bass_utils import run_bass_kernel_spmd

B, D, U = 8192, 1024, 1024
NCORES = 8
BS = B // NCORES  # per-core batch rows
F = 512           # moving-operand cols per matmul (one PSUM bank of fp32)


def build_nc(bs=BS, d=D, u=U):
    """Build the per-core SPMD Bass program.

    DRAM parameter layouts (host prepares these):
      xh   [KO, 128, bs] fp16   : [x|h]^T, contraction on (KO, partition)
      w    [JB, KO2, 128, 2, 4, 128] fp16 :
           w[j,ko2,p,e,g,n] = W_all[(ko2*2+e)*128+p, (g*JB+j)*128+n]
      bias [128, NT] fp32       : bias[p, t] = b_all[t*128+p]
      ct   [JB, 128, bs] fp16   : c_tm1^T unit-blocks
      h_out/c_out [JB, 128, bs] fp16 : h^T / c^T unit-blocks
    """
    kdim = d + u
    KO = kdim // 128    # contraction 128-blocks
    KO2 = KO // 2       # two contraction blocks per weight tile (2KB lines)
    JB = u // 128       # unit blocks per gate
    NT = 4 * u // 128   # total n-tiles (4 gates)
    BH = bs // F        # moving chunks per xh tile

    f32 = mybir.dt.float32
    f16 = mybir.dt.float16
    SIG = mybir.ActivationFunctionType.Sigmoid
    TANH = mybir.ActivationFunctionType.Tanh

    nc = bacc.Bacc("TRN2", target_bir_lowering=False, debug=False)

    xh = nc.dram_tensor("xh", [KO, 128, bs], f16, kind="ExternalInput").ap()
    w = nc.dram_tensor("w", [JB, KO2, 128, 2, 4, 128], f16, kind="ExternalInput").ap()
    bia = nc.dram_tensor("bias", [128, NT], f32, kind="ExternalInput").ap()
    ct = nc.dram_tensor("ct", [JB, 128, bs], f16, kind="ExternalInput").ap()
    ho = nc.dram_tensor("h_out", [JB, 128, bs], f16, kind="ExternalOutput").ap()
    co = nc.dram_tensor("c_out", [JB, 128, bs], f16, kind="ExternalOutput").ap()

    # f gate first so t1 = f*ct can issue early (the very last group instead
    # uses c~,f,i,o so only act(o) -> h mul -> DMA trails the final matmul)
    G_ORDER = (1, 0, 3, 2)

    with tile.TileContext(nc) as tc, ExitStack() as ctx:
        xh_pool = ctx.enter_context(tc.tile_pool(name="xh", bufs=1))
        w_pool = ctx.enter_context(tc.tile_pool(name="w", bufs=2 * KO2))
        bias_pool = ctx.enter_context(tc.tile_pool(name="bias", bufs=1))
        ct_pool = ctx.enter_context(tc.tile_pool(name="ct", bufs=2))
        gate_pool = ctx.enter_context(tc.tile_pool(name="gates", bufs=2))
        out_pool = ctx.enter_context(tc.tile_pool(name="outs", bufs=2))
        psum_pool = ctx.enter_context(tc.tile_pool(name="psum", bufs=8, space="PSUM"))

        bias_sb = bias_pool.tile([128, NT], f32, tag="bias")
        nc.sync.dma_start(bias_sb[:], bia[:])

        # HAM warm-up: ~4us of dependency-free matmuls on garbage SBUF while
        # the first real tiles are still in flight, so the PE clock gate is
        # already at 8/8 when the real stream starts.
        warm_sb = gate_pool.tile([128, 128], f16, tag="warm")
        nc.vector.memset(warm_sb[:], 0.0)
        warm_ps = psum_pool.tile([128, F], f32, tag="ps", name="warm_ps")
        for _ in range(90):
            nc.tensor.matmul(
                warm_ps[:, :64], lhsT=warm_sb[:], rhs=warm_sb[:, :64],
                start=True, stop=True,
            )

        def load_ct(j):
            t = ct_pool.tile([128, bs], f16, tag="ct")
            nc.sync.dma_start(t[:], ct[j])
            return t

        def load_wk(j, ko2):
            t = w_pool.tile([128, 2, 4, 128], f16, tag="wk", name=f"wk_{j}_{ko2}")
            nc.sync.dma_start(t[:], w[j, ko2])
            return t

        # Startup: interleave j=0 weights with the xh stream so the PE can
        # start after the first ~0.5MB instead of the full prefix.
        xh_sb = []
        wk_by_j = {0: []}
        ct_by_j = {}
        for ko2 in range(KO2):
            wk_by_j[0].append(load_wk(0, ko2))
            for e in range(2):
                t = xh_pool.tile([128, bs], f16, tag=f"xh{2 * ko2 + e}")
                nc.sync.dma_start(t[:], xh[2 * ko2 + e])
                xh_sb.append(t)
        # ct0 after the whole chase stream: it is only needed by j=0's
        # epilogue (~40us), and mid-stream it stalls the PE's ko chase
        ct_by_j[0] = load_ct(0)

        def act_gate(j, g, ps, gtile=None, gsl=slice(0, F)):
            if gtile is None:
                gtile = gate_pool.tile([128, F], f32, tag=f"g{g}")
            idx = g * JB + j
            func = TANH if g == 2 else SIG
            nc.scalar.activation(
                gtile[:, gsl], ps[:, gsl], func, bias=bias_sb[:, idx : idx + 1]
            )
            return gtile

        def epilogue(j, bh, gt, ct_sb, h_out, c_out, do_dma=False):
            gsl = slice(0, F)
            bsl = slice(bh * F, (bh + 1) * F)
            t1 = gate_pool.tile([128, F], f32, tag="t1")
            nc.vector.tensor_mul(t1[:], gt[1][:, gsl], ct_sb[:, bsl])
            t2 = gate_pool.tile([128, F], f32, tag="t2")
            nc.vector.tensor_mul(t2[:], gt[0][:, gsl], gt[2][:, gsl])
            nc.vector.tensor_add(c_out[:, bsl], t1[:], t2[:])
            tct = gate_pool.tile([128, F], f32, tag="tct")
            nc.scalar.activation(tct[:], c_out[:, bsl], TANH)
            nc.vector.tensor_mul(h_out[:, bsl], gt[3][:, gsl], tct[:])
            if do_dma:
                nc.sync.dma_start(ho[j][:, bsl], h_out[:, bsl])
                nc.scalar.dma_start(co[j][:, bsl], c_out[:, bsl])

        for j in range(JB):
            if j + 1 < JB:
                wk_by_j[j + 1] = [load_wk(j + 1, ko2) for ko2 in range(KO2)]
                ct_by_j[j + 1] = load_ct(j + 1)
            wk = wk_by_j.pop(j)
            ct_sb = ct_by_j.pop(j)
            h_out = out_pool.tile([128, bs], f16, tag="h")
            c_out = out_pool.tile([128, bs], f16, tag="c")
            gt = {}
            if j == 0:
                # ko-major chase in two waves: 7 groups while xh streams in
                # (max cushion against early-DMA jitter), then the last c~
                # group, so j=1's first group only waits on the first
                # wave-1 activation.
                wave1 = [(g, bh) for g in G_ORDER[:3] for bh in range(BH)]
                wave1.append((2, 0))
                ps = {
                    g: [
                        psum_pool.tile([128, F], f32, tag="ps", name=f"ps0_{g}_{bh}")
                        for bh in range(BH)
                    ]
                    for g in G_ORDER
                }
                for ko in range(KO):
                    for g, bh in wave1:
                        nc.tensor.matmul(
                            ps[g][bh][:],
                            lhsT=wk[ko // 2][:, ko % 2, g, :],
                            rhs=xh_sb[ko][:, bh * F : (bh + 1) * F],
                            start=(ko == 0),
                            stop=(ko == KO - 1),
                        )
                for ko in range(KO):
                    nc.tensor.matmul(
                        ps[2][1][:],
                        lhsT=wk[ko // 2][:, ko % 2, 2, :],
                        rhs=xh_sb[ko][:, F : 2 * F],
                        start=(ko == 0),
                        stop=(ko == KO - 1),
                    )
                for g in G_ORDER:
                    gb = []
                    for bh in range(BH):
                        gtile = act_gate(j, g, ps[g][bh])
                        gb.append(gtile)
                    gt[g] = gb
                for bh in range(BH):
                    epilogue(j, bh, [gt[g][bh] for g in range(4)], ct_sb, h_out, c_out)
                nc.sync.dma_start(ho[j][:], h_out[:])
                nc.scalar.dma_start(co[j][:], c_out[:])
            else:
                # bh-major: bh=0's gates+epilogue fully overlap bh=1's
                # matmuls, so only bh=1's final chain trails the last MM.
                last = j == JB - 1
                for bh in range(BH):
                    # Last group of the whole kernel: gate order c~,f,i,o so
                    # c (and its DMA) completes during the o-gate matmuls and
                    # the post-last-MM chain is just act(o) -> h mul -> DMA.
                    gorder = (2, 1, 0, 3) if (last and bh == BH - 1) else G_ORDER
                    gtb = {}
                    for g in gorder:
                        psb = psum_pool.tile(
                            [128, F], f32, tag="ps", name=f"ps_{g}_{bh}"
                        )
                        for ko in range(KO):
                            nc.tensor.matmul(
                                psb[:],
                                lhsT=wk[ko // 2][:, ko % 2, g, :],
                                rhs=xh_sb[ko][:, bh * F : (bh + 1) * F],
                                start=(ko == 0),
                                stop=(ko == KO - 1),
                            )
                        if last and bh == BH - 1 and g == 3:
                            gtb[g] = psb  # act chunked below, after the chain
                        else:
                            gtb[g] = act_gate(j, g, psb)
                        if last and bh == BH - 1 and g == 0:
                            # c-chain runs here, overlapped by the o-gate MMs
                            bsl = slice(bh * F, (bh + 1) * F)
                            t1 = gate_pool.tile([128, F], f32, tag="t1")
                            nc.vector.tensor_mul(t1[:], gtb[1][:], ct_sb[:, bsl])
                            t2 = gate_pool.tile([128, F], f32, tag="t2")
                            nc.vector.tensor_mul(t2[:], gtb[0][:], gtb[2][:])
                            nc.vector.tensor_add(c_out[:, bsl], t1[:], t2[:])
                            nc.scalar.dma_start(co[j][:, bsl], c_out[:, bsl])
                            tct = gate_pool.tile([128, F], f32, tag="tct")
                            nc.scalar.activation(tct[:], c_out[:, bsl], TANH)
                    if not (last and bh == BH - 1):
                        epilogue(
                            j, bh, [gtb[g] for g in range(4)], ct_sb, h_out, c_out,
                            do_dma=(last and bh == 0),
                        )
                        if not last and bh == BH - 1:
                            nc.sync.dma_start(ho[j][:], h_out[:])
                            nc.scalar.dma_start(co[j][:], c_out[:])
                    else:
                        # post-last-MM: act(o) + h mul + DMA in 256-col chunks
                        oo = gate_pool.tile([128, F], f32, tag="g3b")
                        for ci in range(2):
                            wd = F // 2
                            gsl = slice(ci * wd, (ci + 1) * wd)
                            act_gate(j, 3, gtb[3], gtile=oo, gsl=gsl)
                            bsl = slice(bh * F + ci * wd, bh * F + (ci + 1) * wd)
                            nc.vector.tensor_mul(
                                h_out[:, bsl], oo[:, gsl], tct[:, gsl]
                            )
                            nc.sync.dma_start(ho[j][:, bsl], h_out[:, bsl])

    nc.compile()
    return nc


def pack_shared(inputs):
    """Weight + bias device arrays (replicated on every core)."""
    d, u = inputs["W_i"].shape[0], inputs["W_i"].shape[1]
    kdim = d + u
    KO = kdim // 128
    KO2 = KO // 2
    NT = 4 * u // 128
    JB = u // 128
    Wx = np.concatenate(
        [inputs["W_i"], inputs["W_f"], inputs["W_c"], inputs["W_o"]], axis=1
    )
    Uh = np.concatenate(
        [inputs["U_i"], inputs["U_f"], inputs["U_c"], inputs["U_o"]], axis=1
    )
    W_all = np.concatenate([Wx, Uh], axis=0)  # [kdim, 4u]
    # w_dev[j, ko2, p, e, g, n] = W_all[(ko2*2+e)*128+p, (g*JB+j)*128+n]
    w_dev = np.ascontiguousarray(
        W_all.reshape(KO2, 2, 128, 4, JB, 128).transpose(4, 0, 2, 1, 3, 5)
    ).astype(np.float16)
    b_all = np.concatenate(
        [inputs["b_i"], inputs["b_f"], inputs["b_c"], inputs["b_o"]]
    )  # [4u]
    b_dev = np.ascontiguousarray(b_all.reshape(NT, 128).T).astype(np.float32)
    return w_dev, b_dev


def pack_core(x_i, h_i, c_i):
    """Per-core shard arrays."""
    bs = x_i.shape[0]
    d, u = x_i.shape[1], h_i.shape[1]
    KO = (d + u) // 128
    JB = u // 128
    xh_t = np.concatenate([x_i, h_i], axis=1).T  # [kdim, bs]
    xh_dev = np.ascontiguousarray(xh_t.reshape(KO, 128, bs)).astype(np.float16)
    ct_dev = np.ascontiguousarray(c_i.T.reshape(JB, 128, bs)).astype(np.float16)
    return xh_dev, ct_dev


_NC_CACHE = {}


def _get_nc():
    key = (BS, D, U)
    if key not in _NC_CACHE:
        _NC_CACHE[key] = build_nc()
    return _NC_CACHE[key]


def _run(inputs, trace=False):
    x = np.asarray(inputs["inputs"], np.float32)
    h = np.asarray(inputs["h_tm1"], np.float32)
    c = np.asarray(inputs["c_tm1"], np.float32)
    w_dev, b_dev = pack_shared(inputs)
    in_maps = []
    for i in range(NCORES):
        sl = slice(i * BS, (i + 1) * BS)
        xh_dev, ct_dev = pack_core(x[sl], h[sl], c[sl])
        in_maps.append({"xh": xh_dev, "w": w_dev, "bias": b_dev, "ct": ct_dev})
    nc = _get_nc()
    res = run_bass_kernel_spmd(nc, in_maps, list(range(NCORES)), trace=trace)
    u = U
    h_full = np.empty((B, u), np.float32)
    c_full = np.empty((B, u), np.float32)
    for i in range(NCORES):
        sl = slice(i * BS, (i + 1) * BS)
        h_full[sl] = res.results[i]["h_out"].astype(np.float32).reshape(u, BS).T
        c_full[sl] = res.results[i]["c_out"].astype(np.float32).reshape(u, BS).T
    return (h_full, c_full), res


def kernel(**inputs):
    out, _ = _run(inputs, trace=False)
    return out
